# revision 22
# baseline (speedup 1.0000x reference)
"""AWQ 4-bit quantized linear (x @ dequant(qweight)) on 8 NeuronCores.

Column-parallel tensor sharding: each core owns OUT_F/8 = 1376 output
columns; x is replicated (pre-transposed to [in, tok] so the contraction
dim lands on SBUF partitions).

Layout trick: qweight int32 columns are viewed as int16 on host (pure
reinterpret), and the device unpacks 4 nibbles per int16 word with
CONTIGUOUS DVE writes into an "s-major" permuted column order
o' = s*344 + j  (original column o = 8*(j//2) + 4*(j%2) + s).
Scales are host-permuted to match; the device computes y in permuted
column order and the host un-permutes y at the end.

Per-core kernel:
  - prologue: unpack zero-points (4 DVE ops), cast fp16, park z||s rows
    in a DRAM scratch so per-group rows can be DMA-broadcast across
    partitions in one transfer per k-block;
  - dequant all 32 k-blocks into resident SBUF W tiles: 4x DVE nibble
    unpack (int16), ACT int16->fp16 cast, DVE subtract broadcast zeros,
    DVE multiply broadcast scales. sub/mul emission is software-
    pipelined one k-block apart so dependent DVE ops never run
    back-to-back (avoids the DVE pipe-drain stall);
  - pass1 (interleaved into dequant emission): y partial over k-blocks
    0..HK for all tokens, PSUM-accumulated in a single 3-bank tile,
    spilled fp32 straight from PSUM to DRAM by one DMA (no ACT evict);
  - pass2: accumulate k-blocks HK..32 in PSUM, DVE-fused
    (psum + spill) -> fp16 eviction, DMA out.
Bulk x loads and spill writes issue from the GpSimd queue so the
latency-critical qweight/broadcast loads on the Sync queue never sit
behind megabyte transfers. Output gathered host-side (concat shards +
un-permute columns).
"""

import ctypes

import numpy as np

try:  # un-wedge a stale axon tunnel left by a previously killed run
    _axon = ctypes.CDLL("/opt/axon/libaxon_pjrt.so")
    _axon.axon_reset.restype = ctypes.c_int64
    _axon.axon_reset()
except OSError:
    pass

import concourse.mybir as mybir
import concourse.tile as tile
from concourse import bacc
from concourse._compat import axon_active
from concourse.tile_rust import add_dep_helper

FP16 = mybir.dt.float16
FP32 = mybir.dt.float32
I16 = mybir.dt.int16

P = 128
N_CORES = 8
IN_F = 4096
OUT_F = 11008
GROUP = 128            # quant group size == k-block size
NG = IN_F // GROUP     # 32 k-blocks
TOK = 2 * 2048         # tokens

OSH = OUT_F // N_CORES     # 1376 out columns per core
OPACK = OSH // 8           # 172 packed int32 columns per core
OP16 = OSH // 4            # 344 int16 words per core

SHIFT = mybir.AluOpType.logical_shift_right
AND = mybir.AluOpType.bitwise_and

# device column layout: o' = s*OP16 + j holds original column
# o = 8*(j//2) + 4*(j%2) + s
_J = np.arange(OP16)
IDX = np.concatenate([8 * (_J // 2) + 4 * (_J % 2) + s for s in range(4)])
INV = np.empty(OSH, dtype=np.int64)
INV[IDX] = np.arange(OSH)

CHUNKS = ((0, 512), (512, 512), (1024, OSH - 1024))


def build_program(tok=TOK, in_f=IN_F, osh=OSH, tok_macro=256, hk=5):
    """Emit the SPMD per-core program. Returns the compiled Bacc module."""
    ng = in_f // GROUP
    op16 = osh // 4
    assert tok % tok_macro == 0 and tok_macro % P == 0

    nc = bacc.Bacc("TRN2", target_bir_lowering=False, debug=not axon_active())
    xt = nc.declare_dram_parameter("xt", [in_f, tok], FP16, isOutput=False)
    qw = nc.declare_dram_parameter("qw", [in_f, op16], I16, isOutput=False)
    qz = nc.declare_dram_parameter("qz", [ng, op16], I16, isOutput=False)
    sc = nc.declare_dram_parameter("sc", [ng, osh], FP16, isOutput=False)
    y = nc.declare_dram_parameter("y", [tok, osh], FP16, isOutput=True)
    zs2 = nc.dram_tensor("zs2scratch", [ng, 2, osh], FP16)
    ysp = nc.dram_tensor("yspill", [tok, osh], FP16)

    n_macro = tok // tok_macro
    tt_per_macro = tok_macro // P
    n_units = n_macro * tt_per_macro

    with tile.TileContext(nc) as tc:
        # ---- prologue: unpack zero-points to fp16, park z||s in DRAM ----
        with tc.tile_pool(name="prpool", bufs=1) as prpool:
            qzt = prpool.tile([ng, op16], I16)
            nc.sync.dma_start(qzt[:], qz[:])
            swrite = nc.sync.dma_start(zs2[:, 1, :], sc[:])
            z16i = prpool.tile([ng, osh], I16)
            for s in range(4):
                nc.vector.tensor_scalar(z16i[:, s * op16:(s + 1) * op16],
                                        qzt[:], 4 * s, 15, SHIFT, AND)
            z16f = prpool.tile([ng, osh], FP16)
            nc.scalar.copy(z16f[:], z16i[:])
            # same sync queue as the zsb broadcasts: queue FIFO + the
            # explicit dep below both order the RAW on zs2
            zwrite = nc.sync.dma_start(zs2[:, 0, :], z16f[:])

        with (
            tc.tile_pool(name="wpool", bufs=1) as wpool,
            tc.tile_pool(name="xpool", bufs=2) as xpool,
            tc.tile_pool(name="qwpool", bufs=4) as qwpool,
            tc.tile_pool(name="bpool", bufs=3) as bpool,
            tc.tile_pool(name="ipool", bufs=3) as ipool,
            tc.tile_pool(name="fpool", bufs=4) as fpool,
            tc.tile_pool(name="stpool", bufs=3) as stpool,
            tc.tile_pool(name="y1pool", bufs=2) as y1pool,
            tc.tile_pool(name="pspool", bufs=2, space="PSUM") as pspool,
        ):
            wts = []
            ywrites = {}

            def load_x_macro(m, k0, k1, tag, eng=None):
                # one DMA for all k-blocks [k0,k1) of macro m:
                # DRAM [(k1-k0)*P, tok_macro] region -> SBUF [P, ...]
                t0 = m * tok_macro
                nk = k1 - k0
                xtile = xpool.tile([P, nk * tok_macro], FP16, tag=tag,
                                   name=f"{tag}m{m}")
                src = xt[k0 * P:k1 * P, t0:t0 + tok_macro]
                (eng or nc.gpsimd).dma_start(
                    xtile[:].rearrange("p (a t) -> p a t", a=nk),
                    src.rearrange("(a p) t -> p a t", p=P))
                return xtile

            def pass1_units(pre):
                for m in range(n_macro):
                    t0 = m * tok_macro
                    xtile = pre.pop(m, None)
                    if xtile is None:
                        xtile = load_x_macro(m, 0, hk, "xp1")
                    for tt in range(tt_per_macro):
                        ps = pspool.tile([P, 1536], FP32, tag="ps",
                                         name="ps")
                        for k in range(hk):
                            lhs = xtile[:, k * tok_macro + tt * P:
                                        k * tok_macro + (tt + 1) * P]
                            for j, (o0, on) in enumerate(CHUNKS):
                                nc.tensor.matmul(
                                    ps[:, o0:o0 + on], lhs,
                                    wts[k][:, o0:o0 + on],
                                    start=(k == 0), stop=(k == hk - 1))
                        s16 = stpool.tile([P, osh], FP16, tag="s16")
                        for j, (o0, on) in enumerate(CHUNKS):
                            nc.scalar.copy(s16[:, o0:o0 + on],
                                           ps[:, o0:o0 + on])
                        r0 = t0 + tt * P
                        ywrites[r0] = nc.gpsimd.dma_start(
                            ysp[r0:r0 + P, :], s16[:])
                        yield

            # prefetch the first qweight rows before the dequant loop; the
            # first two pass-1 x macros are loaded inside it (after g=0/g=2)
            # so their bulk transfers don't starve the latency-critical
            # qweight/broadcast chain on the shared DMA engines
            pre = {}
            qwpre = {}
            for g in range(3):
                qwt = qwpool.tile([P, op16], I16, tag="qw")
                nc.sync.dma_start(qwt[:], qw[g * P:(g + 1) * P, :])
                qwpre[g] = qwt
            p1 = pass1_units(pre)
            emitted = 0
            pend = None  # (tmp, sb, g) whose mul is not yet emitted

            # ---- dequant all k-blocks, pass-1 interleaved by pacing ----
            for g in range(ng):
                qwt = qwpre.pop(g, None)
                if qwt is None:
                    qwt = qwpool.tile([P, op16], I16, tag="qw")
                    nc.sync.dma_start(qwt[:], qw[g * P:(g + 1) * P, :])
                zsb = bpool.tile([P, 2 * osh], FP16, tag="zsb")
                zsread = nc.sync.dma_start(
                    zsb[:].rearrange("p (a o) -> p a o", a=2),
                    zs2[g:g + 1, :, :].to_broadcast((P, 2, osh)))
                add_dep_helper(zsread.ins, zwrite.ins, sync=True,
                               reason="zs2 RAW z")
                add_dep_helper(zsread.ins, swrite.ins, sync=True,
                               reason="zs2 RAW s")

                iw16i = ipool.tile([P, osh], I16, tag="iw16i")
                for s in range(4):
                    nc.vector.tensor_scalar(iw16i[:, s * op16:(s + 1) * op16],
                                            qwt[:], 4 * s, 15, SHIFT, AND)

                # emit the pending mul of k-block g-1 between this block's
                # unpack and sub so dependent DVE ops never run adjacent
                if pend is not None:
                    ptmp, psb, pg = pend
                    wt = wpool.tile([P, osh], FP16, tag=f"w{pg}",
                                    name=f"w{pg}")
                    nc.vector.tensor_mul(wt[:], ptmp[:], psb)
                    wts.append(wt)
                # mixed-dtype sub: int16 nibbles minus fp16 zeros -> fp16
                tmp = fpool.tile([P, osh], FP16, tag="tmp")
                nc.vector.tensor_sub(tmp[:], iw16i[:], zsb[:, :osh])
                pend = (tmp, zsb[:, osh:], g)

                if g == 0:
                    pre[0] = load_x_macro(0, 0, hk, "xp1", eng=nc.scalar)
                elif g == 2:
                    pre[1] = load_x_macro(1, 0, hk, "xp1", eng=nc.scalar)

                if g > hk:
                    want = max(0, min((g - hk) * n_units // (ng - hk - 6),
                                      n_units))
                    while emitted < want:
                        next(p1)
                        emitted += 1
            ptmp, psb, pg = pend
            wt = wpool.tile([P, osh], FP16, tag=f"w{pg}", name=f"w{pg}")
            nc.vector.tensor_mul(wt[:], ptmp[:], psb)
            wts.append(wt)
            for _ in p1:
                emitted += 1

            # ---- pass 2: accumulate k>=hk, add spill, emit y ----
            for m in range(n_macro):
                t0 = m * tok_macro
                xtile = load_x_macro(m, hk, ng, "xp2")
                for tt in range(tt_per_macro):
                    r0 = t0 + tt * P
                    y1t = y1pool.tile([P, osh], FP16, tag="y1")
                    # gpsimd queue = same queue as the pass-1 spill writes
                    yread = nc.gpsimd.dma_start(y1t[:], ysp[r0:r0 + P, :])
                    add_dep_helper(yread.ins, ywrites[r0].ins, sync=True,
                                   reason="yspill RAW")
                    ps = pspool.tile([P, 1536], FP32, tag="ps", name="ps")
                    for k in range(hk, ng):
                        a = k - hk
                        lhs = xtile[:, a * tok_macro + tt * P:
                                    a * tok_macro + (tt + 1) * P]
                        for j, (o0, on) in enumerate(CHUNKS):
                            nc.tensor.matmul(
                                ps[:, o0:o0 + on], lhs,
                                wts[k][:, o0:o0 + on],
                                start=(k == hk), stop=(k == ng - 1))
                    st = stpool.tile([P, osh], FP16, tag="st")
                    for j, (o0, on) in enumerate(CHUNKS):
                        nc.vector.tensor_add(st[:, o0:o0 + on],
                                             ps[:, o0:o0 + on],
                                             y1t[:, o0:o0 + on])
                        nc.scalar.dma_start(y[r0:r0 + P, o0:o0 + on],
                                            st[:, o0:o0 + on])

    nc.compile()
    return nc


_PROGRAM = None

# test-harness hooks (unused by the grading path)
TRACE = False
TRACE_KWARGS = {}
LAST_RESULT = None


def _get_program():
    global _PROGRAM
    if _PROGRAM is None:
        _PROGRAM = build_program()
    return _PROGRAM


def kernel(x, qweight, qzeros, scales):
    from concourse.bass_utils import run_bass_kernel_spmd

    x = np.asarray(x)
    qweight = np.asarray(qweight)
    qzeros = np.asarray(qzeros)
    scales = np.asarray(scales)

    xt = np.ascontiguousarray(x.reshape(TOK, IN_F).T)
    in_maps = []
    for c in range(N_CORES):
        qw_c = np.ascontiguousarray(
            qweight[:, c * OPACK:(c + 1) * OPACK]).view(np.int16)
        qz_c = np.ascontiguousarray(
            qzeros[:, c * OPACK:(c + 1) * OPACK]).view(np.int16)
        sc_c = np.ascontiguousarray(
            scales[:, c * OSH:(c + 1) * OSH][:, IDX])
        in_maps.append({"xt": xt, "qw": qw_c, "qz": qz_c, "sc": sc_c})

    nc = _get_program()
    res = run_bass_kernel_spmd(nc, in_maps, list(range(N_CORES)),
                               trace=TRACE, **TRACE_KWARGS)
    global LAST_RESULT
    LAST_RESULT = res
    y = np.concatenate(
        [res.results[i]["y"][:, INV] for i in range(N_CORES)], axis=1)
    return y.reshape(x.shape[0], x.shape[1], OUT_F)
